# revision 5
# baseline (speedup 1.0000x reference)
"""DecoderRNN (2-layer LSTM teacher-forced decode + vocab projection) on 8 trn2 cores.

Strategy:
- LSTM is replicated on all 8 cores (recurrence is latency-bound; collectives
  per step would dominate). Projection is vocab-sharded 8 ways; host concats.
- All matmuls in fp32r (full PE rate at N>=256, ~2^-13 rounding).
- Per layer: batched input-transform pass (Gx = X @ Wx.T + b, via DRAM), then
  the 65-step recurrence with weights streamed from SBUF as the moving operand
  (stationary = h.T chunks, rebuilt each step via PE transposes). The Gx[t]
  term is accumulated into PSUM with a K=32 identity matmul.

Shapes: B=32, T=64, V=50257, E=H=512, L=2.
"""
import os
import numpy as np
import ml_dtypes

import concourse.bass as bass
import concourse.mybir as mybir
import concourse.tile as tile
from concourse import bacc
from concourse.bass_utils import run_bass_kernel_spmd
from contextlib import ExitStack

P = 128
B = 32
T = 64
S = T + 1                  # LSTM steps (features + T caption embeddings)
NT = S * B                 # 2080 tokens through the LSTM
NTOK = T * B               # 2048 projected tokens (steps 1..64)
E = 512
H = 512
G = 4 * H                  # 2048 gates
KC = E // P                # 4 contraction chunks
V = 50257
NCORES = 8
V8 = 6283                  # ceil(V / 8); padded vocab = 8 * 6283 = 50264
NSL = (V8 + 511) // 512    # 13 vocab slices of <=512
V8P = NSL * 512            # 6656, wo DRAM padded so every slice DMA is full

f32 = mybir.dt.float32
f32r = mybir.dt.float32r
bf16 = mybir.dt.bfloat16
i32 = mybir.dt.int32

Sigmoid = mybir.ActivationFunctionType.Sigmoid
Tanh = mybir.ActivationFunctionType.Tanh

_NC_CACHE = None
LAST_RESULTS = None


def _bcast_ap(src_ap, nrows):
    """Partition-broadcast a [1, ...] DRAM AP to nrows partitions."""
    return bass.AP(tensor=src_ap.tensor, offset=src_ap.offset,
                   ap=[[0, nrows]] + [list(d) for d in src_ap.ap[1:]])


def _emit_batched(nc, tc, pools, x_src, wx, b_bc, gx_dram, feats_d, cidx_d, embed_d, id128):
    """Phase A+B: gather X token-tiles, transpose, Gx = X @ Wx.T + b -> DRAM.

    x_src is None for layer 0 (gather from embeddings) or h0T tile for layer 1.
    """
    misc, bpsum = pools
    ntiles = (NT + P - 1) // P  # 17
    for m in range(ntiles):
        rows = min(P, NT - m * P)
        if x_src is None:
            xg = misc.tile([P, E], f32r, tag="xg")
            if m == 0:
                nidx = P - B
                idx_t = misc.tile([P, 1], i32, tag="idx")
                nc.sync.dma_start(out=idx_t[0:nidx, :], in_=cidx_d[0:nidx, :])
                nc.sync.dma_start(out=xg[0:B, :], in_=feats_d[:])
                nc.gpsimd.indirect_dma_start(
                    out=xg[B:P, :], out_offset=None, in_=embed_d[:],
                    in_offset=bass.IndirectOffsetOnAxis(ap=idx_t[0:nidx, :], axis=0))
            else:
                idx_t = misc.tile([P, 1], i32, tag="idx")
                nc.sync.dma_start(out=idx_t[0:rows, :],
                                  in_=cidx_d[m * P - B:m * P - B + rows, :])
                nc.gpsimd.indirect_dma_start(
                    out=xg[0:rows, :], out_offset=None, in_=embed_d[:],
                    in_offset=bass.IndirectOffsetOnAxis(ap=idx_t[0:rows, :], axis=0))
            xt = misc.tile([P, KC, P], f32r, tag="xt")
            for k in range(KC):
                tp = bpsum.tile([P, P], f32r, tag="tpb")
                nc.tensor.transpose(out=tp[:, 0:rows], in_=xg[0:rows, bass.ts(k, P)],
                                    identity=id128[0:rows, 0:rows])
                nc.vector.tensor_copy(xt[:, k, 0:rows], tp[:, 0:rows])
            lhsT = lambda k: xt[:, k, 0:rows]
        else:
            lhsT = lambda k: x_src[:, k, m * P:m * P + rows]

        for n in range(4):
            ps = bpsum.tile([P, 512], f32, tag="bmm")
            for k in range(KC):
                nc.tensor.matmul(out=ps[0:rows, :], lhsT=lhsT(k),
                                 rhs=wx[:, k, bass.ts(n, 512)],
                                 start=(k == 0), stop=(k == KC - 1))
            gxw = misc.tile([P, 512], f32r, tag="gxw")
            nc.vector.tensor_add(gxw[0:rows, :], ps[0:rows, :], b_bc[0:rows, bass.ts(n, 512)])
            nc.sync.dma_start(out=gx_dram[m * P:m * P + rows, bass.ts(n, 512)],
                              in_=gxw[0:rows, :])


def _emit_recurrence(nc, tc, pools, gx_dram, wh, hT_all, id32, cpool, lname):
    """65-step LSTM recurrence for one layer. Writes hT_all[:, k, t*B:(t+1)*B]."""
    gxpool, misc, gpsum, tpsum = pools
    c = cpool.tile([B, H], f32, tag=f"c{lname}")
    nc.gpsimd.memset(c[:], 0.0)
    for t in range(S):
        gxr = gxpool.tile([B, G], f32r, tag="gxr")
        nc.sync.dma_start(out=gxr[:], in_=gx_dram[t * B:(t + 1) * B, :])
        psg = []
        for n in range(4):
            ps = gpsum.tile([B, 512], f32, tag="gates")
            if t > 0:
                for k in range(KC):
                    nc.tensor.matmul(out=ps[:], lhsT=hT_all[:, k, (t - 1) * B:t * B],
                                     rhs=wh[:, k, bass.ts(n, 512)],
                                     start=(k == 0), stop=False)
            nc.tensor.matmul(out=ps[:], lhsT=id32, rhs=gxr[:, bass.ts(n, 512)],
                             start=(t == 0), stop=True)
            psg.append(ps)
        sig_i = misc.tile([B, 512], f32, tag="sig_i")
        nc.scalar.activation(sig_i[:], psg[0][:], Sigmoid)
        sig_f = misc.tile([B, 512], f32, tag="sig_f")
        nc.scalar.activation(sig_f[:], psg[1][:], Sigmoid)
        tanh_g = misc.tile([B, 512], f32, tag="tanh_g")
        nc.scalar.activation(tanh_g[:], psg[2][:], Tanh)
        sig_o = misc.tile([B, 512], f32, tag="sig_o")
        nc.scalar.activation(sig_o[:], psg[3][:], Sigmoid)
        # c = sig_f * c + sig_i * tanh_g   (c starts at 0 so t=0 keeps 0 for f-term)
        nc.vector.tensor_mul(c[:], c[:], sig_f[:])
        ig = misc.tile([B, 512], f32, tag="ig")
        nc.vector.tensor_mul(ig[:], sig_i[:], tanh_g[:])
        nc.vector.tensor_add(c[:], c[:], ig[:])
        tanh_c = misc.tile([B, 512], f32, tag="tanh_c")
        nc.scalar.activation(tanh_c[:], c[:], Tanh)
        h = misc.tile([B, H], f32r, tag="h")
        nc.vector.tensor_mul(h[:], sig_o[:], tanh_c[:])
        for k in range(KC):
            tp = tpsum.tile([P, B], f32r, tag="tph")
            nc.tensor.transpose(out=tp[:], in_=h[:, bass.ts(k, P)], identity=id32)
            nc.vector.tensor_copy(hT_all[:, k, t * B:(t + 1) * B], tp[:])


def _build_nc():
    nc = bacc.Bacc("TRN2", target_bir_lowering=False, num_devices=NCORES)

    feats_d = nc.dram_tensor("feats", [B, E], f32r, kind="ExternalInput")
    cidx_d = nc.dram_tensor("cidx", [NTOK, 1], i32, kind="ExternalInput")
    embed_d = nc.dram_tensor("embed", [V, E], f32r, kind="ExternalInput")
    wx0_d = nc.dram_tensor("wx0", [P, KC, G], f32r, kind="ExternalInput")
    wh0_d = nc.dram_tensor("wh0", [P, KC, G], f32r, kind="ExternalInput")
    wx1_d = nc.dram_tensor("wx1", [P, KC, G], f32r, kind="ExternalInput")
    wh1_d = nc.dram_tensor("wh1", [P, KC, G], f32r, kind="ExternalInput")
    b0_d = nc.dram_tensor("b0", [1, G], f32, kind="ExternalInput")
    b1_d = nc.dram_tensor("b1", [1, G], f32, kind="ExternalInput")
    wo_d = nc.dram_tensor("wo", [P, KC, V8P], f32r, kind="ExternalInput")
    bo_d = nc.dram_tensor("bo", [1, V8], bf16, kind="ExternalInput")
    id128_d = nc.dram_tensor("id128", [P, P], f32r, kind="ExternalInput")

    logits_d = nc.dram_tensor("logits", [NTOK, V8], f32, kind="ExternalOutput")

    gx0_dram = nc.dram_tensor("gx0_i", [NT, G], f32r)
    gx1_dram = nc.dram_tensor("gx1_i", [NT, G], f32r)

    with tile.TileContext(nc) as tc, ExitStack() as ctx:
        const = ctx.enter_context(tc.tile_pool(name="const", bufs=1))
        wpool = ctx.enter_context(tc.tile_pool(name="wpool", bufs=2))
        state = ctx.enter_context(tc.tile_pool(name="state", bufs=1))
        misc = ctx.enter_context(tc.tile_pool(name="misc", bufs=2))
        nl = ctx.enter_context(tc.tile_pool(name="nl", bufs=1))
        proj = ctx.enter_context(tc.tile_pool(name="proj", bufs=2))

        id128 = const.tile([P, P], f32r)
        nc.sync.dma_start(out=id128, in_=id128_d[:])
        id32 = id128[0:B, 0:B]

        b_bc = const.tile([P, G], f32)
        nc.sync.dma_start(out=b_bc, in_=_bcast_ap(b0_d[:], P))
        bo_bc = const.tile([P, V8], bf16)
        nc.sync.dma_start(out=bo_bc, in_=_bcast_ap(bo_d[:], P))

        # ---------------- Layer 0 ----------------
        wx0 = wpool.tile([P, KC, G], f32r, tag="w")
        nc.sync.dma_start(out=wx0, in_=wx0_d[:])
        with tc.tile_pool(name="bpsum", bufs=3, space="PSUM") as bpsum:
            _emit_batched(nc, tc, (misc, bpsum), None, wx0, b_bc, gx0_dram,
                          feats_d, cidx_d, embed_d, id128)

        wh0 = wpool.tile([P, KC, G], f32r, tag="w")
        nc.sync.dma_start(out=wh0, in_=wh0_d[:])
        with tc.tile_pool(name="h0t", bufs=1) as h0tp:
            h0T = h0tp.tile([P, KC, NT], f32r)
            with tc.tile_pool(name="gpsum0", bufs=4, space="PSUM") as gpsum, \
                 tc.tile_pool(name="tpsum0", bufs=2, space="PSUM") as tpsum:
                _emit_recurrence(nc, tc, (misc, nl, gpsum, tpsum), gx0_dram, wh0, h0T,
                                 id32, state, "0")

            # ---------------- Layer 1 batched x-part ----------------
            wx1 = wpool.tile([P, KC, G], f32r, tag="w")
            nc.sync.dma_start(out=wx1, in_=wx1_d[:])
            nc.sync.dma_start(out=b_bc, in_=_bcast_ap(b1_d[:], P))
            with tc.tile_pool(name="bpsum1", bufs=3, space="PSUM") as bpsum1:
                _emit_batched(nc, tc, (misc, bpsum1), h0T, wx1, b_bc, gx1_dram,
                              feats_d, cidx_d, embed_d, id128)

        # ---------------- Layer 1 recurrence ----------------
        wh1 = wpool.tile([P, KC, G], f32r, tag="w")
        nc.sync.dma_start(out=wh1, in_=wh1_d[:])
        with tc.tile_pool(name="h1t", bufs=1) as h1tp:
            h1T = h1tp.tile([P, KC, NT], f32r)
            with tc.tile_pool(name="gpsum1", bufs=4, space="PSUM") as gpsum, \
                 tc.tile_pool(name="tpsum1", bufs=2, space="PSUM") as tpsum:
                _emit_recurrence(nc, tc, (misc, nl, gpsum, tpsum), gx1_dram, wh1, h1T,
                                 id32, state, "1")

            # ---------------- Projection (vocab-sharded) ----------------
            with tc.tile_pool(name="ppsum", bufs=4, space="PSUM") as ppsum:
                for n in range(NSL):
                    ns = min(512, V8 - n * 512)
                    wo = proj.tile([P, KC, 512], f32r, tag="wo")
                    nc.sync.dma_start(out=wo[:], in_=wo_d[:, :, bass.ts(n, 512)])
                    for m in range(NTOK // P):
                        pp = ppsum.tile([P, 512], f32, tag="pp")
                        for k in range(KC):
                            # always matmul the full 512 (f32r needs aligned
                            # free size); garbage tail columns are dropped
                            nc.tensor.matmul(
                                out=pp[:],
                                lhsT=h1T[:, k, B + m * P:B + (m + 1) * P],
                                rhs=wo[:, k, :],
                                start=(k == 0), stop=(k == KC - 1))
                        ob = proj.tile([P, 512], f32, tag="ob")
                        nc.vector.tensor_add(ob[:, 0:ns], pp[:, 0:ns],
                                             bo_bc[:, n * 512:n * 512 + ns])
                        nc.sync.dma_start(
                            out=logits_d[m * P:(m + 1) * P, n * 512:n * 512 + ns],
                            in_=ob[:, 0:ns])

    nc.finalize()
    return nc


def _pack_wT(w):
    """[out_dim, in_dim=512] -> [128, KC, out_dim] fp32 (w.T chunked for SBUF)."""
    wt = np.ascontiguousarray(w.T.astype(np.float32))          # [512, out]
    return np.ascontiguousarray(
        wt.reshape(KC, P, w.shape[0]).transpose(1, 0, 2))       # [128, KC, out]


def kernel(features, captions, embed_w, w_ih0, w_hh0, b0,
           w_ih1, w_hh1, b1, w_out, b_out):
    global _NC_CACHE, LAST_RESULTS
    features = np.asarray(features, dtype=np.float32)
    captions = np.asarray(captions)
    embed_w = np.asarray(embed_w, dtype=np.float32)

    cidx = np.ascontiguousarray(
        np.asarray(captions).T.reshape(NTOK, 1).astype(np.int32))

    wx0 = _pack_wT(np.asarray(w_ih0, dtype=np.float32))
    wh0 = _pack_wT(np.asarray(w_hh0, dtype=np.float32))
    wx1 = _pack_wT(np.asarray(w_ih1, dtype=np.float32))
    wh1 = _pack_wT(np.asarray(w_hh1, dtype=np.float32))
    woT = _pack_wT(np.asarray(w_out, dtype=np.float32))         # [128, KC, V]
    woT_pad = np.zeros((P, KC, V8 * NCORES), dtype=np.float32)
    woT_pad[:, :, :V] = woT
    bo_pad = np.zeros((V8 * NCORES,), dtype=np.float32)
    bo_pad[:V] = np.asarray(b_out, dtype=np.float32)

    base = {
        "feats": features,
        "cidx": cidx,
        "embed": embed_w,
        "wx0": wx0, "wh0": wh0, "wx1": wx1, "wh1": wh1,
        "b0": np.asarray(b0, dtype=np.float32).reshape(1, G),
        "b1": np.asarray(b1, dtype=np.float32).reshape(1, G),
        "id128": np.eye(P, dtype=np.float32),
    }
    in_maps = []
    for c in range(NCORES):
        m = dict(base)
        wo_c = np.zeros((P, KC, V8P), dtype=np.float32)
        wo_c[:, :, :V8] = woT_pad[:, :, c * V8:(c + 1) * V8]
        m["wo"] = wo_c
        m["bo"] = bo_pad[c * V8:(c + 1) * V8].reshape(1, V8).astype(ml_dtypes.bfloat16)
        in_maps.append(m)

    if _NC_CACHE is None:
        _NC_CACHE = _build_nc()
    nc = _NC_CACHE

    res = run_bass_kernel_spmd(nc, in_maps, list(range(NCORES)))
    LAST_RESULTS = res

    full = np.concatenate([res.results[c]["logits"] for c in range(NCORES)],
                          axis=1)                                # [2048, 50264]
    logits = full[:, :V].reshape(T, B, V).transpose(1, 0, 2)     # [B, T, V]
    return np.ascontiguousarray(logits)


# revision 7
# speedup vs baseline: 1.1599x; 1.1599x over previous
"""DecoderRNN (2-layer LSTM teacher-forced decode + vocab projection) on 8 trn2 cores.

Strategy (v2):
- LSTM replicated on all 8 cores (recurrence is latency-bound; per-step
  collectives would dominate). Projection vocab-sharded 8 ways; host concats.
- Fully fused window pipeline (4 steps/window): gather+transpose window tokens,
  batched x-transform (bf16), layer-0 steps, batched layer-1 x-transform from
  h0 (bf16), layer-1 steps. Gx windows stay in SBUF; the Gx[t] term enters
  PSUM via strip-indexed K=32 identity matmuls (tile_position row groups).
- Recurrence matmuls in fp32r (error compounds over 65 steps); all single-pass
  matmuls (x-transforms, projection, embeddings) in bf16.
- Final dense bf16 projection phase; bias via K=1 ones-row matmuls.
"""
import numpy as np
import ml_dtypes

import concourse.bass as bass
import concourse.mybir as mybir
import concourse.tile as tile
from concourse import bacc
from concourse.bass_utils import run_bass_kernel_spmd
from contextlib import ExitStack

P = 128
B = 32
T = 64
S = T + 1                  # 65 LSTM steps (features + T caption embeddings)
NT = S * B                 # 2080 tokens through the LSTM
NTOK = T * B               # 2048 projected tokens (steps 1..64)
E = 512
H = 512
G = 4 * H                  # 2048 gates, order [i|f|g|o]
KC = E // P                # 4 contraction chunks
V = 50257
NCORES = 8
V8 = 6283                  # ceil(V / 8); padded vocab = 8 * 6283 = 50264
NSL = (V8 + 511) // 512    # 13 vocab slices of <=512
V8P = NSL * 512            # 6656: wo/bo DRAM padded so every slice is full
NW = (S + 3) // 4          # 17 windows of up to 4 steps

f32 = mybir.dt.float32
f32r = mybir.dt.float32r
bf16 = mybir.dt.bfloat16
i32 = mybir.dt.int32

Sigmoid = mybir.ActivationFunctionType.Sigmoid
Tanh = mybir.ActivationFunctionType.Tanh

_NC_CACHE = None
LAST_RESULTS = None


def _build_nc():
    nc = bacc.Bacc("TRN2", target_bir_lowering=False, num_devices=NCORES)

    feats_d = nc.dram_tensor("feats", [B, E], bf16, kind="ExternalInput")
    cidx_d = nc.dram_tensor("cidx", [NTOK, 1], i32, kind="ExternalInput")
    embed_d = nc.dram_tensor("embed", [V, E], bf16, kind="ExternalInput")
    wh0_d = nc.dram_tensor("wh0", [P, KC, G], f32r, kind="ExternalInput")
    wh1_d = nc.dram_tensor("wh1", [P, KC, G], f32r, kind="ExternalInput")
    wx0_d = nc.dram_tensor("wx0", [P, KC, G], bf16, kind="ExternalInput")
    wx1_d = nc.dram_tensor("wx1", [P, KC, G], bf16, kind="ExternalInput")
    b0_d = nc.dram_tensor("b0", [1, G], bf16, kind="ExternalInput")
    b1_d = nc.dram_tensor("b1", [1, G], bf16, kind="ExternalInput")
    wo_d = nc.dram_tensor("wo", [P, KC, V8P], bf16, kind="ExternalInput")
    bo_d = nc.dram_tensor("bo", [1, V8P], bf16, kind="ExternalInput")
    id128b_d = nc.dram_tensor("id128b", [P, P], bf16, kind="ExternalInput")
    id4_d = nc.dram_tensor("id4", [P, B], f32r, kind="ExternalInput")
    ones_d = nc.dram_tensor("ones", [1, P], bf16, kind="ExternalInput")

    logits_d = nc.dram_tensor("logits", [NTOK, V8], f32, kind="ExternalOutput")

    with tile.TileContext(nc) as tc, ExitStack() as ctx:
        const = ctx.enter_context(tc.tile_pool(name="const", bufs=1))
        wts = ctx.enter_context(tc.tile_pool(name="wts", bufs=1))
        state = ctx.enter_context(tc.tile_pool(name="state", bufs=1))
        misc = ctx.enter_context(tc.tile_pool(name="misc", bufs=2))
        gxp = ctx.enter_context(tc.tile_pool(name="gxp", bufs=2))
        nl = ctx.enter_context(tc.tile_pool(name="nl", bufs=2))
        proj = ctx.enter_context(tc.tile_pool(name="proj", bufs=2))
        gpsum = ctx.enter_context(tc.tile_pool(name="gpsum", bufs=5, space="PSUM"))
        aux = ctx.enter_context(tc.tile_pool(name="aux", bufs=3, space="PSUM"))

        id128b = const.tile([P, P], bf16)
        id4 = const.tile([P, B], f32r)
        ones = const.tile([1, P], bf16)
        b0b = const.tile([1, G], bf16)
        b1b = const.tile([1, G], bf16)
        for t_, d_ in ((id128b, id128b_d), (id4, id4_d), (ones, ones_d),
                       (b0b, b0_d), (b1b, b1_d)):
            nc.sync.dma_start(out=t_, in_=d_[:])
        id32 = id4[0:B, :]

        wh0 = wts.tile([P, KC, G], f32r, tag="wh0")
        wh1 = wts.tile([P, KC, G], f32r, tag="wh1")
        wx0 = wts.tile([P, KC, G], bf16, tag="wx0")
        wx1 = wts.tile([P, KC, G], bf16, tag="wx1")
        for t_, d_ in ((wh0, wh0_d), (wh1, wh1_d), (wx0, wx0_d), (wx1, wx1_d)):
            nc.sync.dma_start(out=t_, in_=d_[:])

        c0 = state.tile([B, H], f32, tag="c0")
        c1 = state.tile([B, H], f32, tag="c1")
        nc.gpsimd.memset(c0[:], 0.0)
        nc.gpsimd.memset(c1[:], 0.0)
        h0r = [state.tile([P, KC, B], f32r, tag=f"h0r{i}", name=f"h0r{i}") for i in range(2)]
        h1r = [state.tile([P, KC, B], f32r, tag=f"h1r{i}", name=f"h1r{i}") for i in range(2)]
        h1b_all = state.tile([P, KC, NT], bf16, tag="h1b")

        def batched_x(lhs_fn, wxb, bb, rows, tag):
            """Gx window = X @ Wx.T + b -> f32r SBUF tile [rows, G]."""
            gx = gxp.tile([P, G], f32r, tag="gx")
            for n in range(4):
                ps = aux.tile([P, 512], f32, tag="aux")
                for k in range(KC):
                    nc.tensor.matmul(out=ps[0:rows, :], lhsT=lhs_fn(k),
                                     rhs=wxb[:, k, bass.ts(n, 512)],
                                     start=(k == 0), stop=False)
                nc.tensor.matmul(out=ps[0:rows, :], lhsT=ones[:, 0:rows],
                                 rhs=bb[:, bass.ts(n, 512)],
                                 start=False, stop=True)
                nc.vector.tensor_copy(gx[0:rows, bass.ts(n, 512)], ps[0:rows, :])
            return gx

        def lstm_step(t, j, gx, wh, hr, c, h_extra_sink):
            """One LSTM step; h transposed into hr[t%2] and h_extra_sink."""
            psg = []
            for n in range(4):
                ps = gpsum.tile([B, 512], f32, tag="gates")
                if t > 0:
                    prev = hr[(t - 1) % 2]
                    for k in range(KC):
                        nc.tensor.matmul(out=ps[:], lhsT=prev[:, k, :],
                                         rhs=wh[:, k, bass.ts(n, 512)],
                                         start=(k == 0), stop=False)
                nc.tensor.matmul(out=ps[:], lhsT=id4[32 * j:32 * (j + 1), :],
                                 rhs=gx[32 * j:32 * (j + 1), bass.ts(n, 512)],
                                 start=(t == 0), stop=True,
                                 tile_position=(32 * j, 0))
                psg.append(ps)
            sig_i = nl.tile([B, 512], f32, tag="sig_i")
            nc.scalar.activation(sig_i[:], psg[0][:], Sigmoid)
            sig_f = nl.tile([B, 512], f32, tag="sig_f")
            nc.scalar.activation(sig_f[:], psg[1][:], Sigmoid)
            tanh_g = nl.tile([B, 512], f32, tag="tanh_g")
            nc.scalar.activation(tanh_g[:], psg[2][:], Tanh)
            sig_o = nl.tile([B, 512], f32, tag="sig_o")
            nc.scalar.activation(sig_o[:], psg[3][:], Sigmoid)
            nc.vector.tensor_mul(c[:], c[:], sig_f[:])
            ig = nl.tile([B, 512], f32, tag="ig")
            nc.vector.tensor_mul(ig[:], sig_i[:], tanh_g[:])
            nc.vector.tensor_add(c[:], c[:], ig[:])
            tanh_c = nl.tile([B, 512], f32, tag="tanh_g")
            nc.scalar.activation(tanh_c[:], c[:], Tanh)
            h = nl.tile([B, H], f32r, tag="h")
            nc.vector.tensor_mul(h[:], sig_o[:], tanh_c[:])
            hdst = hr[t % 2]
            for k in range(KC):
                tp = aux.tile([P, B], f32r, tag="aux")
                nc.tensor.transpose(out=tp[:], in_=h[:, bass.ts(k, P)], identity=id32)
                nc.vector.tensor_copy(hdst[:, k, :], tp[:])
                nc.vector.tensor_copy(h_extra_sink(k, j), tp[:])

        for w in range(NW):
            t0 = 4 * w
            nsteps = min(4, S - t0)
            rows = B * nsteps

            # gather + transpose window tokens (bf16)
            xg = misc.tile([P, E], bf16, tag="xg")
            if w == 0:
                nidx = rows - B
                idx_t = misc.tile([P, 1], i32, tag="idx")
                nc.sync.dma_start(out=idx_t[0:nidx, :], in_=cidx_d[0:nidx, :])
                nc.sync.dma_start(out=xg[0:B, :], in_=feats_d[:])
                nc.gpsimd.indirect_dma_start(
                    out=xg[B:rows, :], out_offset=None, in_=embed_d[:],
                    in_offset=bass.IndirectOffsetOnAxis(ap=idx_t[0:nidx, :], axis=0))
            else:
                idx_t = misc.tile([P, 1], i32, tag="idx")
                nc.sync.dma_start(out=idx_t[0:rows, :],
                                  in_=cidx_d[w * P - B:w * P - B + rows, :])
                nc.gpsimd.indirect_dma_start(
                    out=xg[0:rows, :], out_offset=None, in_=embed_d[:],
                    in_offset=bass.IndirectOffsetOnAxis(ap=idx_t[0:rows, :], axis=0))
            xt = misc.tile([P, KC, P], bf16, tag="xt")
            for k in range(KC):
                tp = aux.tile([P, P], bf16, tag="aux")
                nc.tensor.transpose(out=tp[:, 0:rows], in_=xg[0:rows, bass.ts(k, P)],
                                    identity=id128b[0:rows, 0:rows])
                nc.vector.tensor_copy(xt[:, k, 0:rows], tp[:, 0:rows])

            # layer 0: batched x-transform, then steps
            gx0 = batched_x(lambda k: xt[:, k, 0:rows], wx0, b0b, rows, "gx0")
            h0bw = misc.tile([P, KC, P], bf16, tag="h0bw")
            for j in range(nsteps):
                lstm_step(t0 + j, j, gx0, wh0, h0r, c0,
                          lambda k, jj: h0bw[:, k, B * jj:B * (jj + 1)])

            # layer 1: batched x-transform from h0 window, then steps
            gx1 = batched_x(lambda k: h0bw[:, k, 0:rows], wx1, b1b, rows, "gx1")
            for j in range(nsteps):
                t = t0 + j
                lstm_step(t, j, gx1, wh1, h1r, c1,
                          lambda k, jj, t_=t: h1b_all[:, k, B * t_:B * (t_ + 1)])

        # projection: logits = h1[steps 1..64] @ wo + bo  (bf16, vocab-sharded)
        for n in range(NSL):
            ns = min(512, V8 - n * 512)
            wo = proj.tile([P, KC, 512], bf16, tag="wo")
            nc.sync.dma_start(out=wo[:], in_=wo_d[:, :, bass.ts(n, 512)])
            bo = proj.tile([1, 512], bf16, tag="bo")
            nc.sync.dma_start(out=bo, in_=bo_d[:, bass.ts(n, 512)])
            for m in range(NTOK // P):
                pp = gpsum.tile([P, 512], f32, tag="gates")
                for k in range(KC):
                    nc.tensor.matmul(out=pp[:],
                                     lhsT=h1b_all[:, k, B + m * P:B + (m + 1) * P],
                                     rhs=wo[:, k, :], start=(k == 0), stop=False)
                nc.tensor.matmul(out=pp[:], lhsT=ones[:], rhs=bo[:],
                                 start=False, stop=True)
                ob = proj.tile([P, 512], f32, tag="ob")
                nc.vector.tensor_copy(ob[:, 0:ns], pp[:, 0:ns])
                nc.sync.dma_start(
                    out=logits_d[m * P:(m + 1) * P, n * 512:n * 512 + ns],
                    in_=ob[:, 0:ns])

    nc.finalize()
    return nc


def _pack_wT(w, dtype):
    """[out_dim, 512] -> [128, KC, out_dim] (w.T chunked to SBUF layout)."""
    wt = np.ascontiguousarray(w.T.astype(np.float32))           # [512, out]
    return np.ascontiguousarray(
        wt.reshape(KC, P, w.shape[0]).transpose(1, 0, 2)).astype(dtype)


def kernel(features, captions, embed_w, w_ih0, w_hh0, b0,
           w_ih1, w_hh1, b1, w_out, b_out):
    global _NC_CACHE, LAST_RESULTS
    features = np.asarray(features, dtype=np.float32)
    embed_w = np.asarray(embed_w, dtype=np.float32)

    cidx = np.ascontiguousarray(
        np.asarray(captions).T.reshape(NTOK, 1).astype(np.int32))

    woT = _pack_wT(np.asarray(w_out, dtype=np.float32), np.float32)  # [128,KC,V]
    woT_pad = np.zeros((P, KC, V8 * NCORES), dtype=np.float32)
    woT_pad[:, :, :V] = woT
    bo_pad = np.zeros((V8 * NCORES,), dtype=np.float32)
    bo_pad[:V] = np.asarray(b_out, dtype=np.float32)

    base = {
        "feats": features.astype(ml_dtypes.bfloat16),
        "cidx": cidx,
        "embed": embed_w.astype(ml_dtypes.bfloat16),
        "wh0": _pack_wT(np.asarray(w_hh0, dtype=np.float32), np.float32),
        "wh1": _pack_wT(np.asarray(w_hh1, dtype=np.float32), np.float32),
        "wx0": _pack_wT(np.asarray(w_ih0, dtype=np.float32), ml_dtypes.bfloat16),
        "wx1": _pack_wT(np.asarray(w_ih1, dtype=np.float32), ml_dtypes.bfloat16),
        "b0": np.asarray(b0, dtype=np.float32).reshape(1, G).astype(ml_dtypes.bfloat16),
        "b1": np.asarray(b1, dtype=np.float32).reshape(1, G).astype(ml_dtypes.bfloat16),
        "id128b": np.eye(P, dtype=ml_dtypes.bfloat16),
        "id4": np.tile(np.eye(B, dtype=np.float32), (KC, 1)),
        "ones": np.ones((1, P), dtype=ml_dtypes.bfloat16),
    }
    in_maps = []
    for c in range(NCORES):
        m = dict(base)
        wo_c = np.zeros((P, KC, V8P), dtype=np.float32)
        wo_c[:, :, :V8] = woT_pad[:, :, c * V8:(c + 1) * V8]
        m["wo"] = wo_c.astype(ml_dtypes.bfloat16)
        bo_c = np.zeros((1, V8P), dtype=np.float32)
        bo_c[0, :V8] = bo_pad[c * V8:(c + 1) * V8]
        m["bo"] = bo_c.astype(ml_dtypes.bfloat16)
        in_maps.append(m)

    if _NC_CACHE is None:
        _NC_CACHE = _build_nc()
    nc = _NC_CACHE

    res = run_bass_kernel_spmd(nc, in_maps, list(range(NCORES)))
    LAST_RESULTS = res

    full = np.concatenate([res.results[c]["logits"] for c in range(NCORES)],
                          axis=1)                                # [2048, 50264]
    logits = full[:, :V].reshape(T, B, V).transpose(1, 0, 2)     # [B, T, V]
    return np.ascontiguousarray(logits)
